# revision 29
# baseline (speedup 1.0000x reference)
"""DSNT double-loss kernel for Trainium2 (8 NeuronCores, data-parallel over B).

Reference computation (per heatmap of 512 total = B32 x C16, each 256x256):
  - softmax over the 65536 pixels of `input`; DSNT expected coords
    pred_x = sum(p * xs[w]), pred_y = sum(p * ys[h])
  - argmax of `target` over the 65536 pixels (first index on ties),
    mapped to tanh-range coords (tx, ty)
  - loss = sum over heatmaps of sqrt((tx-pred_x)^2 + (ty-pred_y)^2) / B

Sharding: B=32 split 4 per core -> 64 heatmaps/core. Each heatmap is
[128 partitions, 512 free] on chip, flat pixel = 512*p + c, h = 2p +
(c>=256), w = c % 256.  Row-blocks of 8 heatmaps: T[k] = [128, 8*512]
covers heatmaps 8k..8k+7; flat gather row = 1024*(hm//8) + 8*p + hm%8.

v4 (baseline 114us -> v2 89.6 -> v3 93.2 -> this):
  - fp8e4m3 input (host cast; ~2e-5 rel loss perturbation, gate 2e-2):
    stream = 4MB input + 16MB f32 target per core.
  - FEW, BIG stream DMAs, all on the sync queue: the HWDGE path allows
    only ~8 outstanding DMAs, so v3's 29 transfers made the issue
    window stall and the last chunks trickle in.  v4 streams 1MB input
    pairs and 2MB target row-blocks (19 transfers).
  - stream order puts gather-resolved target blocks early and the
    maxidx-resolved block T7 last: T0 T1 T2 T6a T6b T3 T4 T5a T5b
    T7(x4 quarters).  Argmax groups: hm 0:32 (T0-T3) and 48:56 (T6)
    resolve mid-stream via row-gather + FIND; hm 32:48 (T4,T5)
    triggers after T5b with its gather racing the T7 tail; hm 56:64
    (T7) needs NO gather: per-partition first-index via max_index on
    quarter tiles, then a one-hot PE matmul selects c* across
    partitions (integer-exact).
  - stage-1 DSNT stats accumulate straight into per-group PSUM tiles;
    phase result DMAs ride the scalar queue; the device ships
    per-heatmap (s, Sx, Sy, p*, c*) and the host finishes the O(B*C)
    scalar math.
"""

import numpy as np
from contextlib import ExitStack

import concourse.bass as bass
import concourse.bacc as bacc
import concourse.tile as tile
from concourse import mybir
from concourse.bass_utils import run_bass_kernel_spmd

F32 = mybir.dt.float32
BF16 = mybir.dt.bfloat16
FP8 = mybir.dt.float8e4
U16 = mybir.dt.uint16
I16 = mybir.dt.int16
OP = mybir.AluOpType
AX = mybir.AxisListType
AF = mybir.ActivationFunctionType

B, CH, H, W = 32, 16, 256, 256
NCORES = 8
BPC = B // NCORES          # 4 batches per core
NHM = BPC * CH             # 64 heatmaps per core
P, C = 128, 512            # on-chip heatmap tile shape
NT = 8                     # target row-blocks (8 heatmaps each)
MX0 = 56                   # first maxidx-resolved heatmap

# gather groups: hm0 -> (n, npad); group (32,16) is the late one
GGROUP = {0: (32, 32), 32: (16, 16), 48: (8, 16)}
GROUPN = {0: 32, 32: 16, 48: 8, MX0: 8}


def make_consts():
    p = np.arange(128, dtype=np.float32)
    i64 = np.arange(64, dtype=np.float32)
    ones = np.ones(128, dtype=np.float32)
    bf = mybir.dt.np(BF16)
    rowbase = 1024.0 * (i64 // 8) + (i64 % 8)
    return {
        # stage-1 matmul moving weights (bf16, exactly representable)
        "wE2": np.stack([ones, (4.0 * p - 255.0) / 256.0], 1).astype(bf),
        "wO2": np.stack([ones, (4.0 * p - 253.0) / 256.0], 1).astype(bf),
        # stage-3 weights (fp32): [ones, xs] for the two w halves
        "r3A": np.stack([ones, (2.0 * p - 255.0) / 256.0], 1),
        "r3B": np.stack([ones, (2.0 * p + 1.0) / 256.0], 1),
        "onesc": ones[:, None].copy(),
        "ident": np.eye(128, dtype=np.float32),
        "cpb": np.broadcast_to(p + 65536.0, (64, 128)).copy(),   # p + BIG
        "rowiota": np.broadcast_to(p, (64, 128)).copy(),
        # per-group flat-row bases 1024*(hm//8) + hm%8 (partition-0 aligned)
        "c128i0": rowbase[0:32][:, None].copy(),
        "c128i32": rowbase[32:48][:, None].copy(),
        "c128i48": rowbase[48:56][:, None].copy(),
        "ones648": np.ones((64, 8), dtype=np.float32),
        # wrapped-index builders: R = Mwrap*rowf, idx = PERM128.T @ R
        "Mwrap": (np.arange(64)[:, None] // 16 == np.arange(4)[None, :]).astype(np.float32),
        "PERM128": (np.arange(64)[:, None] % 16 == np.arange(128)[None, :] % 16).astype(np.float32),
    }


CONST_DTYPES = {
    "wE2": BF16, "wO2": BF16, "r3A": F32, "r3B": F32,
    "onesc": F32, "ident": F32, "cpb": F32, "rowiota": F32,
    "c128i0": F32, "c128i32": F32, "c128i48": F32, "ones648": F32,
    "Mwrap": F32, "PERM128": F32,
}


def build_nc():
    nc = bacc.Bacc(
        "TRN2",
        target_bir_lowering=False,
        debug=False,
        enable_asserts=False,
        num_devices=NCORES,
    )
    # input: 4 pair-blocks of [128, 2*8*512] fp8 (16 heatmaps each)
    inp = nc.dram_tensor("input", [4, P, 16 * C], FP8, kind="ExternalInput").ap()
    # target: 8 row-blocks of [128, 8*512] f32
    tgt = nc.dram_tensor("target", [NT, P, 8 * C], F32, kind="ExternalInput").ap()
    cdram = {
        k: nc.dram_tensor(k, list(v.shape), CONST_DTYPES[k], kind="ExternalInput").ap()
        for k, v in make_consts().items()
    }
    out = nc.dram_tensor("res", [NHM, 8], F32, kind="ExternalOutput").ap()

    with ExitStack() as ctx:
        tc = ctx.enter_context(tile.TileContext(nc))
        cpool = ctx.enter_context(tc.tile_pool(name="consts", bufs=1))
        inpool = ctx.enter_context(tc.tile_pool(name="inp", bufs=2))
        tpool = ctx.enter_context(tc.tile_pool(name="tgt", bufs=3))
        epool = ctx.enter_context(tc.tile_pool(name="e", bufs=2))
        spool = ctx.enter_context(tc.tile_pool(name="sb", bufs=1))
        fpool = ctx.enter_context(tc.tile_pool(name="fin", bufs=1))
        stps = ctx.enter_context(tc.tile_pool(name="stps", bufs=1, space="PSUM"))
        phps = ctx.enter_context(tc.tile_pool(name="phps", bufs=1, space="PSUM"))

        # ---- constants to SBUF (scalar/gpsimd queues; sync stays free)
        ct = {}
        for k, v in CONST_DTYPES.items():
            shape = list(make_consts()[k].shape)
            t = cpool.tile(shape, v, tag=f"c_{k}", name=f"c_{k}")
            (nc.scalar if len(ct) % 2 == 0 else nc.gpsimd).dma_start(t[:], cdram[k])
            ct[k] = t

        # warm the gpsimd DGE gather library early
        zidx = spool.tile([128, 4], I16, tag="zidx")
        nc.gpsimd.memset(zidx[:], 0)
        gwarm = spool.tile([128, C], F32, tag="gwarm")
        nc.gpsimd.dma_gather(
            gwarm[:].rearrange("p (o c) -> p o c", o=1),
            tgt.rearrange("k p (n c) -> (k p n) c", c=C),
            zidx[:], num_idxs=64, num_idxs_reg=64, elem_size=C,
        )

        stats_ps = {hm0: stps.tile([P, 4 * n], F32, tag=f"st{hm0}",
                                   name=f"st{hm0}")
                    for hm0, n in GROUPN.items()}
        RM = {hm0: spool.tile([P, n], F32, tag=f"rm{hm0}", name=f"rm{hm0}")
              for hm0, n in GROUPN.items()}
        resphase = {hm0: spool.tile([n, 8], F32, tag=f"res{hm0}",
                                    name=f"res{hm0}")
                    for hm0, n in GROUPN.items()}
        # shared PSUM scratch (bank-granular; groups use slices in turn)
        rmtt = phps.tile([32, 128], F32, tag="rmtt", name="rmtt")
        iwt = phps.tile([128, 2], F32, tag="iwt", name="iwt")
        s12t = phps.tile([32, 3], F32, tag="s12t", name="s12t")
        mxps = phps.tile([128, 9], F32, tag="mxps", name="mxps")
        # maxidx scratch: in_max rows (col 8q = RM value, rest sentinel)
        inmall = spool.tile([128, 64], F32, tag="inmall")
        nc.vector.memset(inmall[:], 2.0)
        CIfull = spool.tile([128, 8], F32, tag="CIfull")

        def hm_group(hm):
            for hm0, n in GROUPN.items():
                if hm0 <= hm < hm0 + n:
                    return hm0, n
            raise AssertionError(hm)

        def stage1(hm, et, base):
            """4 PE matmuls accumulating heatmap hm's DSNT stats."""
            hm0, n = hm_group(hm)
            j = hm - hm0
            ps = stats_ps[hm0]
            nc.tensor.matmul(ps[:, 2 * j:2 * j + 2], et[:, base + 0:base + 128],
                             ct["wE2"][:], start=True, stop=False)
            nc.tensor.matmul(ps[:, 2 * j:2 * j + 2], et[:, base + 256:base + 384],
                             ct["wO2"][:], start=False, stop=True)
            bcol = 2 * n + 2 * j
            nc.tensor.matmul(ps[:, bcol:bcol + 2], et[:, base + 128:base + 256],
                             ct["wE2"][:], start=True, stop=False)
            nc.tensor.matmul(ps[:, bcol:bcol + 2], et[:, base + 384:base + 512],
                             ct["wO2"][:], start=False, stop=True)

        def resolve_core(hm0, n):
            """global max + first partition holding it, from RM[hm0]."""
            rmt = rmtt[0:n, :]
            nc.tensor.transpose(rmt[:], RM[hm0][:], ct["ident"][:])
            mh = fpool.tile([n, 1], F32, tag=f"mh{hm0}", name=f"mh{hm0}")
            nc.vector.reduce_max(mh[:], rmt[:], axis=AX.X)
            mp = fpool.tile([n, 128], F32, tag=f"mp{hm0}", name=f"mp{hm0}")
            nc.vector.tensor_scalar(mp[:], rmt[:], mh[:], None, op0=OP.is_ge)
            selp = fpool.tile([n, 128], F32, tag=f"selp{hm0}", name=f"selp{hm0}")
            nc.vector.scalar_tensor_tensor(selp[:], mp[:], -65536.0,
                                           ct["cpb"][0:n, :],
                                           op0=OP.mult, op1=OP.add)
            pstar = fpool.tile([n, 1], F32, tag=f"ps{hm0}", name=f"ps{hm0}")
            nc.vector.tensor_reduce(pstar[:], selp[:], axis=AX.X, op=OP.min)
            return mh, pstar

        def stage3(hm0, n, pstar):
            """contract the w axis of the stats; write res cols 0:4."""
            sb = spool.tile([P, 4 * n], F32, tag=f"ssb{hm0}", name=f"ssb{hm0}")
            nc.scalar.copy(sb[:], stats_ps[hm0][:])
            s12 = s12t[0:n, :]
            nc.tensor.matmul(s12[:, 0:2], sb[:, 0:2 * n:2], ct["r3A"][:],
                             start=True, stop=False)
            nc.tensor.matmul(s12[:, 0:2], sb[:, 2 * n:4 * n:2], ct["r3B"][:],
                             start=False, stop=True)
            nc.tensor.matmul(s12[:, 2:3], sb[:, 1:2 * n:2], ct["onesc"][:],
                             start=True, stop=False)
            nc.tensor.matmul(s12[:, 2:3], sb[:, 2 * n + 1:4 * n:2],
                             ct["onesc"][:], start=False, stop=True)
            rs = resphase[hm0]
            nc.vector.tensor_copy(rs[:, 0:3], s12[:])
            nc.vector.tensor_copy(rs[:, 3:4], pstar[:])

        def resolve_argmax(hm0):
            """core resolve + gather dispatch for one group (no stage 3)."""
            n, npad = GGROUP[hm0]
            mh, pstar = resolve_core(hm0, n)
            ncol = npad // 16
            rowfx = fpool.tile([npad, 1], F32, tag=f"rx{hm0}", name=f"rx{hm0}")
            if n < npad:
                nc.vector.memset(rowfx[:], -1.0)
            nc.vector.scalar_tensor_tensor(rowfx[0:n, :], pstar[:], 8.0,
                                           ct[f"c128i{hm0}"][:],
                                           op0=OP.mult, op1=OP.add)
            R = fpool.tile([npad, ncol], F32, tag=f"R{hm0}", name=f"R{hm0}")
            nc.vector.tensor_scalar(R[:], ct["Mwrap"][0:npad, 0:ncol],
                                    rowfx[:], None, op0=OP.mult)
            iw = iwt[:, 0:ncol]
            nc.tensor.matmul(iw[:], ct["PERM128"][0:npad, :], R[:],
                             start=True, stop=True)
            idxw = fpool.tile([128, ncol], I16, tag=f"ix{hm0}", name=f"ix{hm0}")
            nc.vector.tensor_copy(idxw[:], iw[:])
            G = fpool.tile([128, C], F32, tag=f"G{hm0}", name=f"G{hm0}")
            nc.gpsimd.dma_gather(
                G[:].rearrange("p (o c) -> p o c", o=1),
                tgt.rearrange("k p (n c) -> (k p n) c", c=C),
                idxw[:], num_idxs=npad, num_idxs_reg=npad, elem_size=C,
            )
            return mh, G, pstar

        def resolve_a(hm0):
            """argmax + stage 3 (requires stage-1 stats complete)."""
            mh, G, pstar = resolve_argmax(hm0)
            stage3(hm0, GGROUP[hm0][0], pstar)
            return mh, G

        def resolve_b(hm0, mh, G, _ps=None):
            """first-column find on the gathered rows; ship results."""
            n, _ = GGROUP[hm0]
            inm = fpool.tile([n, 8], F32, tag=f"in{hm0}", name=f"in{hm0}")
            nc.vector.tensor_scalar(inm[:], ct["ones648"][0:n, :], mh[:],
                                    None, op0=OP.mult)
            ci = fpool.tile([n, 8], U16, tag=f"ci{hm0}", name=f"ci{hm0}")
            nc.vector.max_index(ci[:], inm[:], G[0:n, :])
            nc.vector.tensor_copy(resphase[hm0][:, 4:5], ci[:, 0:1])
            nc.scalar.dma_start(out[hm0:hm0 + n, :], resphase[hm0][:])

        def target_block(tk, half=None):
            """DMA one target row-block (or half) + row maxima."""
            hm0 = [0, 0, 0, 0, 32, 32, 48, MX0][tk]
            if half is None:
                col0 = 8 * (tk - {0: 0, 32: 4, 48: 6}[hm0])
                tt = tpool.tile([P, 8 * C], F32, tag="tt")
                nc.sync.dma_start(tt[:], tgt[tk])
                nc.vector.tensor_reduce(
                    RM[hm0][:, col0:col0 + 8],
                    tt[:].rearrange("p (n c) -> p n c", n=8),
                    axis=AX.X, op=OP.max)
                return tt
            else:
                th = tpool.tile([P, 4 * C], F32, tag=f"t{tk}h{half}", bufs=1,
                                name=f"t{tk}h{half}")
                nc.sync.dma_start(th[:], tgt[tk][:, half * 4 * C:(half + 1) * 4 * C])
                col0 = 8 * (tk - {32: 4, 48: 6}[hm0]) + 4 * half
                nc.vector.tensor_reduce(
                    RM[hm0][:, col0:col0 + 4],
                    th[:].rearrange("p (n c) -> p n c", n=4),
                    axis=AX.X, op=OP.max)
                return th

        def input_pair(pk):
            """DMA one fp8 input pair-block and exp() its two halves."""
            it = inpool.tile([P, 16 * C], FP8, tag="it")
            nc.sync.dma_start(it[:], inp[pk])
            es = []
            for h in range(2):
                et = epool.tile([P, 8 * C], BF16, tag="et")
                nc.scalar.activation(et[:], it[:, h * 8 * C:(h + 1) * 8 * C],
                                     AF.Exp)
                es.append(et)
            return es

        def stage1_block(sk, et):
            """stage 1 for the 8 heatmaps of super-chunk sk."""
            for j in range(8):
                stage1(8 * sk + j, et, j * C)

        # ---- stream: it01 T0 T1 | it23 T2 T6a | it45 T6b T3 | T4
        #              it67(x4) T5a T5b T7(x4 quarters)
        e01 = input_pair(0)
        t0 = target_block(0)
        stage1_block(0, e01[0])
        t1 = target_block(1)
        stage1_block(1, e01[1])

        e23 = input_pair(1)
        t2 = target_block(2)
        stage1_block(2, e23[0])
        t6a = target_block(6, half=0)
        stage1_block(3, e23[1])

        e45 = input_pair(2)
        t6b = target_block(6, half=1)
        t3 = target_block(3)
        stage1_block(4, e45[0])
        # group hm 0:32 complete after T3
        g0 = resolve_a(0)
        t4 = target_block(4)
        stage1_block(5, e45[1])

        # last input block: 4 fine DMAs + exps of [128, 2048]
        e67 = []
        for q in range(4):
            itq = inpool.tile([P, 4 * C], FP8, tag=f"it3{q}", bufs=1,
                              name=f"it3{q}")
            nc.sync.dma_start(itq[:], inp[3][:, q * 4 * C:(q + 1) * 4 * C])
            etq = epool.tile([P, 4 * C], BF16, tag=f"et3{q}", bufs=1,
                             name=f"et3{q}")
            nc.scalar.activation(etq[:], itq[:], AF.Exp)
            e67.append(etq)
        # group hm 48:56: argmax side complete after T6b; stage 3 must
        # wait for this group's stage-1 stats (e67 exps)
        mh48, G48, ps48 = resolve_argmax(48)
        resolve_b(0, *g0)

        t5a = target_block(5, half=0)
        for j in range(4):
            stage1(48 + j, e67[0], j * C)
        for j in range(4):
            stage1(52 + j, e67[1], j * C)
        stage3(48, 8, ps48)
        t5b = target_block(5, half=1)
        resolve_b(48, mh48, G48)

        # T7 quarters: row maxima + per-partition first-index (no gather)
        for qa in range(4):
            th = tpool.tile([P, 2 * C], F32, tag=f"t7q{qa}", bufs=1,
                            name=f"t7q{qa}")
            nc.sync.dma_start(th[:], tgt[7][:, qa * 2 * C:(qa + 1) * 2 * C])
            nc.vector.tensor_reduce(
                RM[MX0][:, 2 * qa:2 * qa + 2],
                th[:].rearrange("p (n c) -> p n c", n=2),
                axis=AX.X, op=OP.max)
            nc.vector.tensor_copy(inmall[:, 16 * qa:16 * qa + 16:8],
                                  RM[MX0][:, 2 * qa:2 * qa + 2])
            for q in range(2):
                qi = 2 * qa + q
                cq = fpool.tile([128, 8], U16, tag=f"cq{qi}", name=f"cq{qi}")
                nc.vector.max_index(cq[:], inmall[:, 8 * qi:8 * qi + 8],
                                    th[:, 512 * q:512 * q + 512])
                nc.vector.tensor_copy(CIfull[:, qi:qi + 1], cq[:, 0:1])

        for j in range(4):
            stage1(56 + j, e67[2], j * C)
        for j in range(4):
            stage1(60 + j, e67[3], j * C)

        # group hm 32:48 (T4 + T5): gather races the T7 tail
        g32 = resolve_a(32)
        resolve_b(32, *g32)

        # ---- maxidx tail for hm 56:64: c* = CIfull[p*, hm] via one-hot
        # PE matmul (integer-exact)
        mh3, pstar3 = resolve_core(MX0, 8)
        oh3 = fpool.tile([8, 128], F32, tag="oh3")
        nc.vector.tensor_scalar(oh3[:], ct["rowiota"][0:8, :], pstar3[:],
                                None, op0=OP.is_equal)
        ohT = mxps[:, 0:8]
        nc.tensor.transpose(ohT[:], oh3[:], ct["ident"][0:8, 0:8])
        ohTs = fpool.tile([128, 8], F32, tag="ohTs")
        nc.vector.tensor_copy(ohTs[:], ohT[:])
        M3 = fpool.tile([128, 8], F32, tag="M3")
        nc.vector.tensor_mul(M3[:], CIfull[:], ohTs[:])
        cst = mxps[0:8, 8:9]
        nc.tensor.matmul(cst[:], M3[:], ct["onesc"][:], start=True, stop=True)
        stage3(MX0, 8, pstar3)
        nc.vector.tensor_copy(resphase[MX0][:, 4:5], cst[:])
        nc.sync.dma_start(out[MX0:MX0 + 8, :], resphase[MX0][:])

    nc.compile()
    return nc


_NC_CACHE = None


def _get_nc():
    global _NC_CACHE
    if _NC_CACHE is None:
        _NC_CACHE = build_nc()
    return _NC_CACHE


def make_in_maps(input, target):
    consts = make_consts()
    in_maps = []
    for i in range(NCORES):
        def shard(x, nper, dt=None):
            nchunk = NHM // nper
            s = x[i * BPC:(i + 1) * BPC].reshape(nchunk, nper, P, C)
            s = np.ascontiguousarray(
                s.transpose(0, 2, 1, 3).reshape(nchunk, P, nper * C))
            return s.astype(dt) if dt is not None else s
        m = {"input": shard(input, 16, mybir.dt.np(FP8)),
             "target": shard(target, 8)}
        m.update(consts)
        in_maps.append(m)
    return in_maps


def kernel(input, target, _trace=False):
    input = np.asarray(input, dtype=np.float32)
    target = np.asarray(target, dtype=np.float32)
    nc = _get_nc()
    in_maps = make_in_maps(input, target)
    r = run_bass_kernel_spmd(nc, in_maps, list(range(NCORES)), trace=_trace)
    # host-side finish on per-heatmap sufficient statistics (O(B*C))
    total = 0.0
    for cres in r.results:
        v = np.asarray(cres["res"], dtype=np.float64)   # [64, 8]
        s, Sx, Sy, pstar, cstar = (v[:, k] for k in range(5))
        px, py = Sx / s, Sy / s
        bsel = (cstar >= 256.0).astype(np.float64)
        wI = cstar - 256.0 * bsel
        hI = 2.0 * pstar + bsel
        tx = (2.0 * (wI + 1.0) - 257.0) / 256.0
        ty = (2.0 * (hI + 1.0) - 257.0) / 256.0
        total += np.sqrt((tx - px) ** 2 + (ty - py) ** 2).sum()
    outv = np.array([total / 32.0], dtype=np.float32)
    if _trace:
        return outv, r
    return outv
